# revision 20
# baseline (speedup 1.0000x reference)
"""Trainium2 Bass kernel for nn_Encoder (GNN message passing encoder).

Computes, for each node b in a batch:
    h[b]  = concat(mean_j feat[neigh[b, j]], feat[nodes[b]]) @ W.T
    out[b] = relu(layernorm(h[b]))          (torch-style unbiased std, eps on std)
returned as [OUT, B] (transposed).

Strategy (8 NeuronCores, data-parallel over the batch):
  - Each core handles B/8 = 8192 nodes; feat_table/W/gamma/beta replicated.
  - Gathers use SWDGE indirect DMA, 128 rows (one per partition) per
    instruction — the only gather granularity walrus supports. The
    16-neighbor sum is folded into the gather chain with the SDMA CCE
    accumulate op (compute_op=add), so no vector-engine reduction at all.
  - Per 128-node tile: PE transposes the combined [128, 512] activations
    (features onto partitions), then 4 accumulating matmuls against W^T
    compute h in PSUM; LayerNorm runs on ACT/DVE with Square+accum_out and
    a fused Relu(scale=rstd) epilogue.
  - The 1/16 neighbor-mean scale is folded into the first 256 rows of W^T.
"""

import os
import sys

sys.path.insert(0, "/opt/trn_rl_repo")
sys.path.insert(0, "/opt/pypackages")

from contextlib import ExitStack

import numpy as np

import concourse.bass as bass
import concourse.tile as tile
from concourse import bacc, mybir
from concourse.bass_utils import run_bass_kernel_spmd
from concourse.masks import make_identity

# Problem constants (hardcoded; kernel.py must be self-contained).
N_NODES, D, OUT, B, K = 100000, 256, 256, 65536, 16
EPS = 1e-6
NCORES = 8
BLOC = B // NCORES  # 8192 nodes per core
P = 128
NT = BLOC // P  # node-tiles per core (64)

# float32 is exact; float16 halves gather traffic (Pool-bound, so default f32).
TABLE_DT = os.environ.get("ENC_TABLE_DT", "float32")
# 'cce': fold neighbor sum into the gather DMAs; 'pe': separate gathers +
# PE identity-matmul reduction (fallback if CCE accumulate misbehaves).
ACC_MODE = os.environ.get("ENC_ACC_MODE", "pe")
GBUFS = int(os.environ.get("ENC_GBUFS", "12"))
GROUP = int(os.environ.get("ENC_GROUP", "1"))


def build_program(
    n_tab=N_NODES,
    nt=NT,
    k=K,
    table_dt_name=TABLE_DT,
    apply_gamma_beta=True,
    acc_mode=ACC_MODE,
    loop_iters=1,
):
    """Build the Bass program for one core (SPMD across cores)."""
    table_dt = getattr(mybir.dt, table_dt_name)
    f32 = mybir.dt.float32
    i32 = mybir.dt.int32

    nc = bacc.Bacc(
        "TRN2",
        target_bir_lowering=False,
        debug=False,
    )
    feat = nc.declare_dram_parameter("feat", [n_tab, D], table_dt, isOutput=False)
    wt = nc.declare_dram_parameter("wt", [2 * D, OUT], table_dt, isOutput=False)
    nodes_t = nc.declare_dram_parameter("nodes_t", [P, nt], i32, isOutput=False)
    neigh_t = nc.declare_dram_parameter("neigh_t", [P, nt * k], i32, isOutput=False)
    if apply_gamma_beta:
        gamma_b = nc.declare_dram_parameter("gamma_b", [P, OUT], f32, isOutput=False)
        beta_b = nc.declare_dram_parameter("beta_b", [P, OUT], f32, isOutput=False)
    out_d = nc.declare_dram_parameter("out", [P * nt, OUT], f32, isOutput=True)

    with tile.TileContext(nc) as tc, ExitStack() as ctx:
        consts = ctx.enter_context(tc.tile_pool(name="consts", bufs=1))
        pool_ns = ctx.enter_context(tc.tile_pool(name="nsum", bufs=GBUFS))
        pool_sf = ctx.enter_context(tc.tile_pool(name="selfg", bufs=GBUFS))
        pool_c = ctx.enter_context(tc.tile_pool(name="combT", bufs=3))
        pool_f = ctx.enter_context(tc.tile_pool(name="f32s", bufs=3))
        pool_sm = ctx.enter_context(tc.tile_pool(name="small", bufs=4))
        psum_t_pool = ctx.enter_context(tc.tile_pool(name="psumT", bufs=2, space="PSUM"))
        psum_h_pool = ctx.enter_context(tc.tile_pool(name="psumH", bufs=2, space="PSUM"))
        if acc_mode == "pe":
            pool_g = ctx.enter_context(tc.tile_pool(name="gth", bufs=4))
            psum_r_pool = ctx.enter_context(
                tc.tile_pool(name="psumR", bufs=2, space="PSUM")
            )

        # --- constants ---
        ident32 = consts.tile([P, P], f32)
        make_identity(nc, ident32[:])
        if table_dt != f32:
            ident = consts.tile([P, P], table_dt)
            nc.vector.tensor_copy(ident[:], ident32[:])
        else:
            ident = ident32

        wt_sb = consts.tile([P, 4 * OUT], table_dt)
        for c in range(4):
            nc.sync.dma_start(
                out=wt_sb[:, c * OUT : (c + 1) * OUT],
                in_=wt[c * P : (c + 1) * P, :],
            )
        nodes_sb = consts.tile([P, nt], i32)
        nc.sync.dma_start(out=nodes_sb[:], in_=nodes_t[:])
        neigh_sb = consts.tile([P, nt * k], i32)
        nc.sync.dma_start(out=neigh_sb[:], in_=neigh_t[:])
        if apply_gamma_beta:
            gamma_sb = consts.tile([P, OUT], f32)
            nc.sync.dma_start(out=gamma_sb[:], in_=gamma_b[:])
            beta_sb = consts.tile([P, OUT], f32)
            nc.sync.dma_start(out=beta_sb[:], in_=beta_b[:])

        def gather_group(tiles):
            """Emit the gathers for a group of tiles j-major, so the Pool
            sequencer's in-order stream interleaves the CCE accumulate
            chains — each chain-link wait is G instructions stale."""
            nsums, selfs = {}, {}
            if acc_mode == "cce":
                for t in tiles:
                    nsums[t] = pool_ns.tile([P, D], table_dt, tag="nsum", name=f"nsum_{t}")
                for j in range(k):
                    for t in tiles:
                        nc.gpsimd.indirect_dma_start(
                            out=nsums[t][:],
                            out_offset=None,
                            in_=feat[:],
                            in_offset=bass.IndirectOffsetOnAxis(
                                ap=neigh_sb[:, t * k + j : t * k + j + 1], axis=0
                            ),
                            compute_op=(
                                mybir.AluOpType.bypass
                                if j == 0
                                else mybir.AluOpType.add
                            ),
                        )
            else:
                gths = {}
                for t in tiles:
                    gths[t] = pool_g.tile([P, k, D], table_dt, tag="gth", name=f"gth_{t}")
                for j in range(k):
                    for t in tiles:
                        nc.gpsimd.indirect_dma_start(
                            out=gths[t][:, j, :],
                            out_offset=None,
                            in_=feat[:],
                            in_offset=bass.IndirectOffsetOnAxis(
                                ap=neigh_sb[:, t * k + j : t * k + j + 1], axis=0
                            ),
                        )
                for t in tiles:
                    psum_r = psum_r_pool.tile([P, D], f32, tag="psum_r")
                    for j in range(k):
                        nc.tensor.matmul(
                            psum_r[:],
                            lhsT=ident[:],
                            rhs=gths[t][:, j, :],
                            start=(j == 0),
                            stop=(j == k - 1),
                        )
                    nsums[t] = pool_ns.tile([P, D], table_dt, tag="nsum", name=f"nsum_{t}")
                    nc.vector.tensor_copy(nsums[t][:], psum_r[:])
            for t in tiles:
                selfs[t] = pool_sf.tile([P, D], table_dt, tag="selfg", name=f"selfg_{t}")
                nc.gpsimd.indirect_dma_start(
                    out=selfs[t][:],
                    out_offset=None,
                    in_=feat[:],
                    in_offset=bass.IndirectOffsetOnAxis(
                        ap=nodes_sb[:, t : t + 1], axis=0
                    ),
                )
            return nsums, selfs

        def tile_compute(t, nsum_src, selfg):
            # --- transpose combined activations onto feature-partitions ---
            psum_t = psum_t_pool.tile([P, 512], table_dt)
            srcs = [
                nsum_src[:, 0:P],
                nsum_src[:, P:D],
                selfg[:, 0:P],
                selfg[:, P:D],
            ]
            for c, src in enumerate(srcs):
                nc.tensor.transpose(psum_t[:, c * P : (c + 1) * P], src, ident[:])
            combT = pool_c.tile([P, 512], table_dt)
            nc.vector.tensor_copy(combT[:], psum_t[:])

            # --- h = combined @ W^T (1/16 mean-scale pre-folded into wt) ---
            psum_h = psum_h_pool.tile([P, OUT], f32)
            for c in range(4):
                nc.tensor.matmul(
                    psum_h[:],
                    lhsT=combT[:, c * P : (c + 1) * P],
                    rhs=wt_sb[:, c * OUT : (c + 1) * OUT],
                    start=(c == 0),
                    stop=(c == 3),
                )

            # --- LayerNorm (torch unbiased std, eps added to std) + ReLU ---
            negsum = pool_sm.tile([P, 1], f32, tag="negsum")
            nc.vector.tensor_reduce(
                negsum[:], psum_h[:], mybir.AxisListType.X, mybir.AluOpType.add,
                negate=True,
            )
            negmean = pool_sm.tile([P, 1], f32, tag="negmean")
            nc.vector.tensor_scalar_mul(negmean[:], negsum[:], 1.0 / OUT)
            xc = pool_f.tile([P, OUT], f32, tag="xc")
            nc.scalar.activation(
                xc[:], psum_h[:], mybir.ActivationFunctionType.Identity,
                bias=negmean[:, 0:1],
            )
            sq = pool_f.tile([P, OUT], f32, tag="sq")
            ss = pool_sm.tile([P, 1], f32, tag="ss")
            nc.scalar.activation(
                sq[:], xc[:], mybir.ActivationFunctionType.Square,
                accum_out=ss[:, 0:1],
            )
            sstd = pool_sm.tile([P, 1], f32, tag="sstd")
            nc.scalar.activation(
                sstd[:], ss[:], mybir.ActivationFunctionType.Sqrt,
                scale=1.0 / (OUT - 1),
            )
            seps = pool_sm.tile([P, 1], f32, tag="seps")
            nc.vector.tensor_scalar_add(seps[:], sstd[:], EPS)
            rstd = pool_sm.tile([P, 1], f32, tag="rstd")
            nc.vector.reciprocal(rstd[:], seps[:])

            y = pool_f.tile([P, OUT], f32, tag="y")
            if apply_gamma_beta:
                xg = pool_f.tile([P, OUT], f32, tag="xg")
                nc.vector.tensor_tensor(
                    xg[:], xc[:], gamma_sb[:], mybir.AluOpType.mult
                )
                xgs = pool_f.tile([P, OUT], f32, tag="xgs")
                nc.scalar.activation(
                    xgs[:], xg[:], mybir.ActivationFunctionType.Copy,
                    scale=rstd[:, 0:1],
                )
                yb = pool_f.tile([P, OUT], f32, tag="yb")
                nc.vector.tensor_tensor(
                    yb[:], xgs[:], beta_sb[:], mybir.AluOpType.add
                )
                nc.vector.tensor_scalar_max(y[:], yb[:], 0.0)
            else:
                nc.scalar.activation(
                    y[:], xc[:], mybir.ActivationFunctionType.Relu,
                    scale=rstd[:, 0:1],
                )

            nc.sync.dma_start(out=out_d[t * P : (t + 1) * P, :], in_=y[:])

        def body():
            for g0 in range(0, nt, GROUP):
                tiles = list(range(g0, min(g0 + GROUP, nt)))
                nsums, selfs = gather_group(tiles)
                for t in tiles:
                    tile_compute(t, nsums[t][:], selfs[t][:])

        if loop_iters > 1:
            with tc.For_i(0, loop_iters, 1):
                body()
        else:
            body()

    nc.finalize()
    return nc


def _pack_indices(nodes_c, neigh_c, k):
    """Node (t, p) -> batch row t*128 + p.
    nodes_t[p, t] = nodes_c[row];  neigh_t[p, t*k + j] = neigh_c[row, j]."""
    b_loc = nodes_c.shape[0]
    nt = b_loc // P
    ndt = nodes_c.reshape(nt, P).T
    ngt = neigh_c.reshape(nt, P, k).transpose(1, 0, 2).reshape(P, nt * k)
    return np.ascontiguousarray(ndt), np.ascontiguousarray(ngt)


_PROG_CACHE = {}


def prepare(feat_table, W, gamma, beta, nodes, neigh_idx):
    """Build (cached) program + per-core input maps."""
    feat_table = np.asarray(feat_table, dtype=np.float32)
    W = np.asarray(W, dtype=np.float32)
    gamma = np.asarray(gamma, dtype=np.float32)
    beta = np.asarray(beta, dtype=np.float32)
    nodes = np.asarray(nodes).astype(np.int32)
    neigh_idx = np.asarray(neigh_idx).astype(np.int32)

    table_np_dt = np.dtype(np.float16 if TABLE_DT == "float16" else np.float32)

    # combined = [neigh_mean ; self]  ->  W^T rows 0:D get the 1/16 fold.
    wt_host = np.ascontiguousarray(W.T).astype(np.float32)
    wt_host[:D] *= 1.0 / K
    wt_host = wt_host.astype(table_np_dt)

    trivial_affine = bool(np.all(gamma == 1.0) and np.all(beta == 0.0))
    apply_gb = not trivial_affine

    key = (TABLE_DT, apply_gb, ACC_MODE)
    if key not in _PROG_CACHE:
        _PROG_CACHE[key] = build_program(
            table_dt_name=TABLE_DT, apply_gamma_beta=apply_gb, acc_mode=ACC_MODE
        )
    nc = _PROG_CACHE[key]

    feat_dev = np.ascontiguousarray(feat_table.astype(table_np_dt))
    in_maps = []
    for c in range(NCORES):
        nodes_c = nodes[c * BLOC : (c + 1) * BLOC]
        neigh_c = neigh_idx[c * BLOC : (c + 1) * BLOC]
        ndt, ngt = _pack_indices(nodes_c, neigh_c, K)
        m = {
            "feat": feat_dev,
            "wt": wt_host,
            "nodes_t": ndt,
            "neigh_t": ngt,
        }
        if apply_gb:
            m["gamma_b"] = np.ascontiguousarray(
                np.broadcast_to(gamma, (P, OUT))
            ).astype(np.float32)
            m["beta_b"] = np.ascontiguousarray(
                np.broadcast_to(beta, (P, OUT))
            ).astype(np.float32)
        in_maps.append(m)

    return nc, in_maps


def assemble(results):
    out = np.empty((OUT, B), dtype=np.float32)
    for c in range(NCORES):
        out[:, c * BLOC : (c + 1) * BLOC] = results[c]["out"].T
    return out


def kernel(feat_table, W, gamma, beta, nodes, neigh_idx):
    nc, in_maps = prepare(feat_table, W, gamma, beta, nodes, neigh_idx)
    res = run_bass_kernel_spmd(nc, in_maps, list(range(NCORES)))
    return assemble(res.results)
